# revision 1
# baseline (speedup 1.0000x reference)
import os

os.environ.setdefault("NEURON_CC_FLAGS", "--auto-cast=none")

import numpy as np
import jax
import jax.numpy as jnp
from functools import partial

GROUPS = 8
GP = 64
K = 64
EPS = 1e-5
N_CORES = 8

jax.config.update("jax_default_matmul_precision", "highest")


def _bn_dist(t, g, b, axes, axis_name):
    # training-mode batchnorm with cross-device batch statistics
    m = jax.lax.pmean(t.mean(axes, keepdims=True), axis_name)
    msq = jax.lax.pmean((t * t).mean(axes, keepdims=True), axis_name)
    v = msq - m * m
    shape = [1] * t.ndim
    shape[1] = -1
    return (t - m) * jax.lax.rsqrt(v + EPS) * g.reshape(shape) + b.reshape(shape)


@partial(jax.pmap, axis_name="i",
         in_axes=(0, None, None, None, None, None, None, None, None, None, None, None))
def _fwd(xn, qkv_w, bn_qkv_g, bn_qkv_b, bn_sim_g, bn_sim_b, bn_out_g, bn_out_b,
         weight, gamma, all_emb_q, all_emb_kv):
    # xn: [C, H, W] for this device's batch element n
    C, H, W = xn.shape
    B = W
    xp = xn.transpose(2, 0, 1)                            # [W, C, H] == [B, C, H]
    qkv = jnp.einsum("oc,bch->boh", qkv_w, xp)
    qkv = _bn_dist(qkv, bn_qkv_g, bn_qkv_b, (0, 2), "i")
    qkv = qkv.reshape(B, GROUPS, 2 * GP, H)
    q = qkv[:, :, : GP // 2]
    k = qkv[:, :, GP // 2 : GP]
    v = qkv[:, :, GP:]

    q_emb = all_emb_q[: GP // 2]
    k_emb = all_emb_q[GP // 2 :]
    v_emb = all_emb_kv

    qr = jnp.einsum("bgci,cij->bgij", q, q_emb)
    kr = jnp.einsum("bgci,cij->bgij", k, k_emb).transpose(0, 1, 3, 2)
    qk = jnp.einsum("bgci,bgcj->bgij", q, k)
    stacked = jnp.concatenate([qk, qr, kr], axis=1)
    stacked = _bn_dist(stacked, bn_sim_g, bn_sim_b, (0, 2, 3), "i")
    sim = jax.nn.softmax(stacked.reshape(B, 3, GROUPS, H, H).sum(axis=1), axis=3)

    sv = jnp.matmul(jnp.einsum("bgij,bgcj->bgci", sim, v), weight)
    sve = jnp.matmul(jnp.einsum("bgij,cij->bgci", sim, v_emb), weight)
    out = jnp.concatenate([sv, sve], axis=-1).reshape(B, 2 * C * 2 // 2, H)
    out = _bn_dist(out, bn_out_g, bn_out_b, (0, 2), "i")
    out = out.reshape(W, C, 2, H).sum(axis=2).transpose(1, 2, 0)  # [C, H, W]
    return xn + gamma * out


def kernel(x, qkv_w, bn_qkv_g, bn_qkv_b, bn_sim_g, bn_sim_b, bn_out_g, bn_out_b,
           weight, relative, gamma, pos_map):
    x = np.asarray(x, np.float32)
    # host precompute of the static relative-position gather
    rel_idx = np.arange(K)[:, None] - np.arange(K)[None, :] + K - 1
    all_emb = np.asarray(relative)[:, rel_idx] + np.asarray(pos_map)  # [2*GP, K, K]
    all_emb_q = all_emb[:GP].astype(np.float32)      # q_emb + k_emb halves
    all_emb_kv = all_emb[GP:].astype(np.float32)     # v_emb

    out = _fwd(x,
               np.asarray(qkv_w, np.float32),
               np.asarray(bn_qkv_g, np.float32), np.asarray(bn_qkv_b, np.float32),
               np.asarray(bn_sim_g, np.float32), np.asarray(bn_sim_b, np.float32),
               np.asarray(bn_out_g, np.float32), np.asarray(bn_out_b, np.float32),
               np.asarray(weight, np.float32),
               np.float32(gamma),
               all_emb_q, all_emb_kv)
    return np.asarray(out, np.float32)


# revision 4
# speedup vs baseline: 67.2007x; 67.2007x over previous
import os

os.environ.setdefault("NEURON_CC_FLAGS", "--auto-cast=none")

import numpy as np
import jax
import jax.numpy as jnp
from functools import partial

GROUPS = 8
GP = 64
K = 64
EPS = 1e-5
N_CORES = 8

jax.config.update("jax_default_matmul_precision", "highest")


def _bn_dist(t, g, b, axes, axis_name):
    # training-mode batchnorm with cross-device batch statistics
    m = jax.lax.pmean(t.mean(axes, keepdims=True), axis_name)
    msq = jax.lax.pmean((t * t).mean(axes, keepdims=True), axis_name)
    v = msq - m * m
    shape = [1] * t.ndim
    shape[1] = -1
    return (t - m) * jax.lax.rsqrt(v + EPS) * g.reshape(shape) + b.reshape(shape)


def _fwd_impl(xn, qkv_w, bn_qkv_g, bn_qkv_b, bn_sim_g, bn_sim_b, bn_out_g, bn_out_b,
              weight, gamma, all_emb_q, all_emb_kv):
    # xn: [C, H, W] for this device's batch element n
    C, H, W = xn.shape
    B = W
    xp = xn.transpose(2, 0, 1)                            # [W, C, H] == [B, C, H]
    qkv = jnp.einsum("oc,bch->boh", qkv_w, xp)
    qkv = _bn_dist(qkv, bn_qkv_g, bn_qkv_b, (0, 2), "i")
    qkv = qkv.reshape(B, GROUPS, 2 * GP, H)
    q = qkv[:, :, : GP // 2]
    k = qkv[:, :, GP // 2 : GP]
    v = qkv[:, :, GP:]

    q_emb = all_emb_q[: GP // 2]
    k_emb = all_emb_q[GP // 2 :]
    v_emb = all_emb_kv

    qr = jnp.einsum("bgci,cij->bgij", q, q_emb)
    kr = jnp.einsum("bgci,cij->bgij", k, k_emb).transpose(0, 1, 3, 2)
    qk = jnp.einsum("bgci,bgcj->bgij", q, k)
    stacked = jnp.concatenate([qk, qr, kr], axis=1)
    stacked = _bn_dist(stacked, bn_sim_g, bn_sim_b, (0, 2, 3), "i")
    sim = jax.nn.softmax(stacked.reshape(B, 3, GROUPS, H, H).sum(axis=1), axis=3)

    sv = jnp.matmul(jnp.einsum("bgij,bgcj->bgci", sim, v), weight)
    sve = jnp.matmul(jnp.einsum("bgij,cij->bgci", sim, v_emb), weight)
    out = jnp.concatenate([sv, sve], axis=-1).reshape(B, 2 * GROUPS * GP, H)
    out = _bn_dist(out, bn_out_g, bn_out_b, (0, 2), "i")
    out = out.reshape(W, C, 2, H).sum(axis=2).transpose(1, 2, 0)  # [C, H, W]
    return xn + gamma * out


_fwd = jax.pmap(
    _fwd_impl, axis_name="i",
    in_axes=(0, None, None, None, None, None, None, None, None, None, None, None))

# variant where every arg carries a leading device axis: lets callers pre-stage
# weights on-device once (device_put_replicated) instead of re-broadcasting
_fwd_all0 = jax.pmap(_fwd_impl, axis_name="i")


def kernel(x, qkv_w, bn_qkv_g, bn_qkv_b, bn_sim_g, bn_sim_b, bn_out_g, bn_out_b,
           weight, relative, gamma, pos_map):
    x = np.asarray(x, np.float32)
    # host precompute of the static relative-position gather
    rel_idx = np.arange(K)[:, None] - np.arange(K)[None, :] + K - 1
    all_emb = np.asarray(relative)[:, rel_idx] + np.asarray(pos_map)  # [2*GP, K, K]
    all_emb_q = all_emb[:GP].astype(np.float32)      # q_emb + k_emb halves
    all_emb_kv = all_emb[GP:].astype(np.float32)     # v_emb

    out = _fwd(x,
               np.asarray(qkv_w, np.float32),
               np.asarray(bn_qkv_g, np.float32), np.asarray(bn_qkv_b, np.float32),
               np.asarray(bn_sim_g, np.float32), np.asarray(bn_sim_b, np.float32),
               np.asarray(bn_out_g, np.float32), np.asarray(bn_out_b, np.float32),
               np.asarray(weight, np.float32),
               np.float32(gamma),
               all_emb_q, all_emb_kv)
    return np.asarray(out, np.float32)
